# revision 15
# baseline (speedup 1.0000x reference)
"""Trainium2 Bass kernel for nn_BCA_17274358465235.

Module: out = x + conv1x1_up( softmax(fx @ fy_up^T) @ fself ) with
fx/fself/fy two-layer 1x1-conv projections (no nonlinearity between the
two layers, so each pair is FOLDED into a single conv on the host),
fy on bilinearly-upsampled y.  B=4, CX=256, CY=512, CM=64, H=W=64
(N=4096 tokens), HY=WY=32.

Sharding: 8 cores = batch(4) x query-row-half(2).  Each core holds all
4096 keys and its 2048 query rows.  Per-core data is ROTATED on the host
(own half first) and the key/pixel axis is W-DEINTERLEAVED (even w,
then odd w, within each image row) -- both pure permutations of the key
space that keep the program SPMD-uniform, start each core on data that
streams first, and make every upsample output write contiguous (DVE 2x).
The y-upsample boundary rows (outer clamp + rotation seam) arrive as 4
explicit pad rows so no core-dependent control flow is needed.

x and y are shipped HOST-CAST to fp16 (3.3 MB/core input total), so all
projection matmuls run at 1 cycle/col with no on-device casts.  fy2/fx2
live in band-split tiles to avoid coarse-granularity false hazards.
The upsample algebra is scale-absorbed: fyH' = 4*fyH is built with one
scalar_tensor_tensor per band (3c[h] + c[h+-1]) and the 4x is folded
into the 0.75/0.25 taps of the W pass (0.1875/0.0625).

Engine split:  PE: convs + fself^T stream (data-stationary matmuls) +
sim (fp16) + PV (bf16, PSUM accum; ones-column of fself^T produces the
softmax denominator Z free) + up-proj + 1/Z partition-broadcast (K=1
matmul).  ACT: exact exp for ~81% of key chunks, u1 copies.  DVE:
Schraudolph fast-exp (one tensor_scalar to int16 whose bit pattern is
the bf16 exp approximation) for the rest, psum evacuations, upsample
adds, u2 copies, 1/Z, residual adds.  Pool: third DMA queue only.
"""
import sys

for _p in ("/opt/pypackages", "/opt/trn_rl_repo"):
    if _p not in sys.path:
        sys.path.insert(0, _p)

import numpy as np

import concourse.bacc as bacc
import concourse.mybir as mybir
import concourse.tile as tile
from concourse.bass_utils import run_bass_kernel_spmd

F32 = mybir.dt.float32
F16 = mybir.dt.float16
BF16 = mybir.dt.bfloat16
I16 = mybir.dt.int16
EXP = mybir.ActivationFunctionType.Exp
COPY = mybir.ActivationFunctionType.Copy
MUL = mybir.AluOpType.mult
ADD = mybir.AluOpType.add

B, CX, CY, CM = 4, 256, 512, 64
H = W = 64
N = H * W              # 4096 tokens
NH = N // 2            # 2048 query rows per core
KC = N // 128          # 32 key chunks

LN2 = 0.6931471805599453
A16 = 128.0 / LN2                 # Schraudolph scale (int16/bf16 variant)
B16 = 127.0 * 128.0 - 5.5         # Schraudolph offset, C tuned for min-max rel err
# units (h*32 + i) whose exp runs as Schraudolph on DVE (~19%)
SCHR = frozenset(u for u in range(64) if u % 16 in (5, 10, 15))
NPRE = 10                         # fself chunks emitted before the attention loop

_CACHE = {}


def _build(debug=False):
    nc = bacc.Bacc("TRN2", target_bir_lowering=False, debug=False,
                   enable_asserts=False)

    # ---- DRAM I/O (per-core, host-rotated + w-deinterleaved, fp16) ----
    xs16 = nc.dram_tensor("xs16", [128, 8192], F16, kind="ExternalInput").ap()
    yb16 = nc.dram_tensor("yb16", [128, 4096], F16, kind="ExternalInput").ap()
    ybp16 = nc.dram_tensor("ybp16", [128, 512], F16, kind="ExternalInput").ap()
    w16 = nc.dram_tensor("w16", [128, 768], F16, kind="ExternalInput").ap()
    bxy = nc.dram_tensor("bxy", [64, 2], F32, kind="ExternalInput").ap()
    out = nc.dram_tensor("out", [128, 4096], F32, kind="ExternalOutput").ap()
    if debug:
        d_fy2 = nc.dram_tensor("d_fy2", [128, 4096], F32, kind="ExternalOutput").ap()
        d_fx2 = nc.dram_tensor("d_fx2", [128, 2048], F32, kind="ExternalOutput").ap()
        d_fself = nc.dram_tensor("d_fself", [128, 2080], F32, kind="ExternalOutput").ap()
        d_sim0 = nc.dram_tensor("d_sim0", [128, 1024], F32, kind="ExternalOutput").ap()
        d_et0 = nc.dram_tensor("d_et0", [128, 1024], F32, kind="ExternalOutput").ap()
        d_scaled = nc.dram_tensor("d_scaled", [65, 2048], F32, kind="ExternalOutput").ap()

    with tile.TileContext(nc) as tc:
        with tc.tile_pool(name="sbW", bufs=1) as sbW, \
             tc.tile_pool(name="sbM", bufs=1) as sbM:
            # ---- long-lived SBUF ----
            t_xs = sbM.tile([128, 8192], F16)
            t_yb = sbM.tile([128, 4096], F16)
            t_ybp = sbM.tile([128, 512], F16)
            fy2t = [sbM.tile([128, 1024], F16, name=f"fy2_{k}") for k in range(4)]
            fx2a = sbM.tile([128, 1024], F16)
            fx2b = sbM.tile([128, 1024], F16)
            fselfT = sbM.tile([128, KC * 65], BF16)
            scaled = sbM.tile([65, 2048], F16)
            fycb = sbM.tile([64, 1152], F16)    # c + by; 1024:1152 = pads
            fyH = sbM.tile([64, 2048], F16)     # 4x-scaled H-upsample
            u1 = sbM.tile([64, 2048], F16)      # 0.1875 * fyH
            u2 = sbM.tile([64, 2048], F16)      # 0.0625 * fyH
            t_w16 = sbW.tile([128, 768], F16)
            t_bxy = sbW.tile([64, 2], F32)
            fsv = fselfT[:].rearrange("p (k c) -> p k c", c=65)

            # ---- input DMAs, multi-engine issue, critical first ----
            nc.sync.dma_start(t_w16[:], w16[:])
            nc.sync.dma_start(t_xs[:, 0:2048], xs16[:, 0:2048])
            nc.sync.dma_start(t_xs[:, 2048:4096], xs16[:, 2048:4096])
            nc.sync.dma_start(t_bxy[:], bxy[:])
            nc.scalar.dma_start(t_ybp[:], ybp16[:])
            nc.scalar.dma_start(t_yb[:, 0:2048], yb16[:, 0:2048])
            nc.scalar.dma_start(t_yb[:, 2048:4096], yb16[:, 2048:4096])
            nc.gpsimd.dma_start(t_xs[:, 4096:6144], xs16[:, 4096:6144])
            nc.gpsimd.dma_start(t_xs[:, 6144:8192], xs16[:, 6144:8192])

            # warm the ACT exp table; constants
            t_dum = sbW.tile([1, 32], F32)
            nc.vector.memset(t_dum[:], 0.0)
            t_dum2 = sbW.tile([1, 32], F32)
            nc.scalar.activation(t_dum2[:], t_dum[:], EXP)
            nc.vector.memset(fsv[:, :, 0], 1.0)

            # ---- engine helper closures ----
            def fx_blk(psP, q):
                dst = fx2a if q < 2 else fx2b
                p = psP.tile([64, 512], F32, tag="p", bufs=3, name=f"p_fx{q}")
                for a in range(2):
                    nc.tensor.matmul(
                        p[:], t_w16[:, a * 64:(a + 1) * 64],
                        t_xs[:, q * 1024 + a * 512:q * 1024 + a * 512 + 512],
                        start=(a == 0), stop=(a == 1))
                nc.vector.tensor_scalar_add(
                    dst[0:64, (q % 2) * 512:(q % 2) * 512 + 512], p[:],
                    t_bxy[:, 0:1])

            def fy_piece(psP, pp):
                p = psP.tile([64, 512], F32, tag="p", bufs=3, name=f"p_fy{pp}")
                for c in range(4):
                    nc.tensor.matmul(
                        p[:], t_w16[:, 256 + c * 64:256 + (c + 1) * 64],
                        t_yb[:, pp * 2048 + c * 512:pp * 2048 + c * 512 + 512],
                        start=(c == 0), stop=(c == 3))
                nc.vector.tensor_scalar_add(fycb[:, pp * 512:(pp + 1) * 512],
                                            p[:], t_bxy[:, 1:2])

            def fy_pad(psP):
                p = psP.tile([64, 128], F32, tag="ppad", bufs=1, name="p_pad")
                for c in range(4):
                    nc.tensor.matmul(p[:], t_w16[:, 256 + c * 64:256 + (c + 1) * 64],
                                     t_ybp[:, c * 128:(c + 1) * 128],
                                     start=(c == 0), stop=(c == 3))
                nc.vector.tensor_scalar_add(fycb[:, 1024:1152], p[:],
                                            t_bxy[:, 1:2])

            t1 = fycb[:, 0:1024].rearrange("p (h w) -> p h w", h=32)
            fe = fyH[:].rearrange("p (h two w) -> p h two w", h=32, two=2)
            PRE = slice(1024, 1056)
            POST = slice(1056, 1088)
            SEAMA = slice(1088, 1120)
            SEAMB = slice(1120, 1152)
            STT = nc.vector.scalar_tensor_tensor

            def hband_a():
                # fyH' = 4 * (0.75 c[h] + 0.25 c[h']) = 3 c[h] + c[h']
                STT(fe[:, 1:16, 0, :], t1[:, 1:16, :], 3.0, t1[:, 0:15, :], MUL, ADD)
                STT(fe[:, 0:15, 1, :], t1[:, 0:15, :], 3.0, t1[:, 1:16, :], MUL, ADD)
                STT(fe[:, 0, 0, :], t1[:, 0, :], 3.0, fycb[:, PRE], MUL, ADD)

            def hband_b():
                STT(fe[:, 15, 1, :], t1[:, 15, :], 3.0, fycb[:, SEAMA], MUL, ADD)
                STT(fe[:, 16, 0, :], t1[:, 16, :], 3.0, fycb[:, SEAMB], MUL, ADD)
                STT(fe[:, 17:32, 0, :], t1[:, 17:32, :], 3.0, t1[:, 16:31, :], MUL, ADD)
                STT(fe[:, 16:31, 1, :], t1[:, 16:31, :], 3.0, t1[:, 17:32, :], MUL, ADD)
                STT(fe[:, 31, 1, :], t1[:, 31, :], 3.0, fycb[:, POST], MUL, ADD)

            def u1u2(c0, c1):
                nc.scalar.activation(u1[:, c0:c1], fyH[:, c0:c1], COPY, scale=0.1875)
                nc.vector.tensor_scalar_mul(u2[:, c0:c1], fyH[:, c0:c1], 0.0625)

            u1v = u1[:].rearrange("p (h w) -> p h w", h=64)
            u2v = u2[:].rearrange("p (h w) -> p h w", h=64)
            fyHv = fyH[:].rearrange("p (h w) -> p h w", h=64)
            WBANDS = (slice(0, 16), slice(16, 32), slice(32, 48), slice(48, 64))

            def wband(k):
                hs = WBANDS[k]
                # w-deinterleaved: col = (h - hs.start)*64 + par*32 + wq
                fwk = fy2t[k][0:64, :].rearrange("p (h par w) -> p h par w",
                                                 h=16, par=2)
                hv = slice(hs.start, hs.stop)
                nc.vector.tensor_scalar_mul(fwk[:, :, 0, 0], fyHv[:, hv, 0], 0.25)
                nc.vector.tensor_add(fwk[:, :, 0, 1:32], u1v[:, hv, 1:32],
                                     u2v[:, hv, 0:31])
                nc.vector.tensor_add(fwk[:, :, 1, 0:31], u1v[:, hv, 0:31],
                                     u2v[:, hv, 1:32])
                nc.vector.tensor_scalar_mul(fwk[:, :, 1, 31], fyHv[:, hv, 31], 0.25)
                nc.vector.tensor_copy(fy2t[k][64:128, :], fy2t[k][0:64, :])

            fs_ps = {}

            def fself_chunk(psFS, i):
                g, s = divmod(i, 4)
                if s == 0:
                    fs_ps[g % 2] = psFS.tile([128, 256], F32, tag="fs",
                                             bufs=2, name=f"fsg{g}")
                p = fs_ps[g % 2]
                for a in range(2):
                    nc.tensor.matmul(
                        p[:, s * 64:(s + 1) * 64],
                        t_xs[:, g * 1024 + a * 512 + s * 128:
                             g * 1024 + a * 512 + s * 128 + 128],
                        t_w16[:, 128 + a * 64:128 + (a + 1) * 64],
                        start=(a == 0), stop=(a == 1))
                if s == 3:
                    src = p[:].rearrange("p (k c) -> p k c", c=64)
                    nc.vector.tensor_copy(fsv[:, g * 4:(g + 1) * 4, 1:65], src)

            def sim_unit(psB, i, h):
                fx2h = fx2a if h == 0 else fx2b
                fyk = fy2t[i // 8]
                c0 = (i % 8) * 128
                ps = psB.tile([128, 1024], F32, tag="sim", bufs=2,
                              name=f"sim{h}_{i}")
                nc.tensor.matmul(ps[:, 0:512], fyk[0:64, c0:c0 + 128],
                                 fx2h[0:64, 0:512], start=True, stop=True)
                nc.tensor.matmul(ps[:, 512:1024], fyk[64:128, c0:c0 + 128],
                                 fx2h[64:128, 512:1024], start=True, stop=True)
                return ps

            def exp_unit(ps, i, h):
                et = sbM.tile([128, 1024], BF16, tag="et",
                              bufs=3 if debug else 4, name=f"et{h}_{i}")
                if debug and i == 0 and h == 0:
                    d0 = sbM.tile([128, 1024], F32)
                    nc.vector.tensor_copy(d0[:], ps[:])
                    nc.sync.dma_start(d_sim0[:], d0[:])
                if h * 32 + i in SCHR:
                    nc.vector.tensor_scalar(et[:].bitcast(I16), ps[:],
                                            A16, B16, MUL, ADD)
                else:
                    nc.scalar.activation(et[:], ps[:], EXP)
                if debug and i == 5 and h == 0:
                    d1 = sbM.tile([128, 1024], F32)
                    nc.vector.tensor_copy(d1[:], et[:])
                    nc.sync.dma_start(d_et0[:], d1[:])
                return et

            def pv_unit(fout, et, i):
                w = fselfT[:, i * 65:(i + 1) * 65]
                nc.tensor.matmul(fout[:, 0:512], w, et[:, 0:512],
                                 start=(i == 0), stop=(i == KC - 1))
                nc.tensor.matmul(fout[:, 512:1024], w, et[:, 512:1024],
                                 start=(i == 0), stop=(i == KC - 1))

            def pre_tail_s(fout, h, s):
                cs = slice(s * 512, (s + 1) * 512)
                invz = sbM.tile([1, 512], F32, tag="zrow", bufs=2,
                                name=f"invz_{h}_{s}")
                nc.vector.reciprocal_approx_fast(invz[:], fout[0:1, cs])
                invzb = sbM.tile([128, 512], F32, tag="izb", bufs=2,
                                 name=f"invzb_{h}_{s}")
                nc.gpsimd.partition_broadcast(invzb[:], invz[:])
                nc.vector.tensor_mul(
                    scaled[:, h * 1024 + s * 512:h * 1024 + (s + 1) * 512],
                    fout[:, cs], invzb[0:65, :])

            def up_block(psC, q, a):
                p = psC.tile([128, 512], F32, tag="up", bufs=2,
                             name=f"p_up_{q}_{a}")
                nc.tensor.matmul(p[:], t_w16[0:65, 512 + a * 128:512 + (a + 1) * 128],
                                 scaled[0:65, q * 512:(q + 1) * 512],
                                 start=True, stop=True)
                outs = sbM.tile([128, 512], F32, tag="os", bufs=4,
                                name=f"outs_{q}_{a}")
                nc.vector.tensor_add(outs[:], p[:],
                                     t_xs[:, q * 1024 + a * 512:
                                          q * 1024 + a * 512 + 512])
                eng = nc.sync if a == 0 else nc.gpsimd
                eng.dma_start(out[:, a * 2048 + q * 512:
                                  a * 2048 + (q + 1) * 512], outs[:])

            # ================= emission =================
            with tc.tile_pool(name="psA0", bufs=1, space="PSUM") as psA0:
                fout0 = psA0.tile([65, 1024], F32, name="fout0")
                with tc.tile_pool(name="psFS", bufs=1, space="PSUM") as psFS:
                    with tc.tile_pool(name="psP", bufs=1, space="PSUM") as psP:
                        fy_piece(psP, 0)
                        fy_pad(psP)
                        hband_a()
                        u1u2(0, 31 * 32)
                        fx_blk(psP, 0)
                        fx_blk(psP, 1)
                        wband(0)
                        nc.vector.tensor_copy(fx2a[64:128, :], fx2a[0:64, :])
                        for i in range(4):
                            fself_chunk(psFS, i)
                        fy_piece(psP, 1)
                        hband_b()
                        u1u2(31 * 32, 2048)
                        wband(1)
                        for i in range(4, 8):
                            fself_chunk(psFS, i)
                        fx_blk(psP, 2)
                        fx_blk(psP, 3)
                        wband(2)
                        nc.vector.tensor_copy(fx2b[64:128, :], fx2b[0:64, :])
                        for i in range(8, NPRE):
                            fself_chunk(psFS, i)
                        wband(3)
                    with tc.tile_pool(name="psB0", bufs=1, space="PSUM") as psB0:
                        sims = {}
                        sims[0] = sim_unit(psB0, 0, 0)
                        sims[1] = sim_unit(psB0, 1, 0)
                        for i in range(KC):
                            if i + NPRE < KC:
                                fself_chunk(psFS, i + NPRE)
                            et = exp_unit(sims.pop(i), i, 0)
                            pv_unit(fout0, et, i)
                            if i + 2 < KC:
                                sims[i + 2] = sim_unit(psB0, i + 2, 0)
                pre_tail_s(fout0, 0, 0)
                pre_tail_s(fout0, 0, 1)

            with tc.tile_pool(name="psA1", bufs=1, space="PSUM") as psA1:
                fout1 = psA1.tile([65, 1024], F32, name="fout1")
                with tc.tile_pool(name="psC", bufs=1, space="PSUM") as psC:
                    with tc.tile_pool(name="psB1", bufs=1, space="PSUM") as psB1:
                        sims = {}
                        sims[0] = sim_unit(psB1, 0, 1)
                        sims[1] = sim_unit(psB1, 1, 1)
                        up_h0 = [(q, a) for q in (0, 1) for a in (0, 1)]
                        for i in range(KC):
                            et = exp_unit(sims.pop(i), i, 1)
                            pv_unit(fout1, et, i)
                            if i + 2 < KC:
                                sims[i + 2] = sim_unit(psB1, i + 2, 1)
                            if i >= 6 and i % 3 == 0 and up_h0:
                                up_block(psC, *up_h0.pop(0))
                        for qa in up_h0:
                            up_block(psC, *qa)
                    pre_tail_s(fout1, 1, 0)
                    up_block(psC, 2, 0)
                    up_block(psC, 2, 1)
                    pre_tail_s(fout1, 1, 1)
                    up_block(psC, 3, 0)
                    up_block(psC, 3, 1)

            if debug:
                st = sbM.tile([128, 4096], F32, name="dbg_fy2")
                for k in range(4):
                    nc.vector.tensor_copy(st[:, k * 1024:(k + 1) * 1024], fy2t[k][:])
                nc.sync.dma_start(d_fy2[:], st[:])
                st2 = sbM.tile([128, 2048], F32, name="dbg_fx2")
                nc.vector.tensor_copy(st2[:, 0:1024], fx2a[:])
                nc.vector.tensor_copy(st2[:, 1024:2048], fx2b[:])
                nc.sync.dma_start(d_fx2[:], st2[:])
                for nm, dst, src, shp in (("c", d_fself, fselfT, [128, 2080]),
                                          ("d", d_scaled, scaled, [65, 2048])):
                    st3 = sbM.tile(shp, F32, name=f"dbg_{nm}")
                    nc.vector.tensor_copy(st3[:], src[:])
                    nc.sync.dma_start(dst[:], st3[:])

    nc.compile()
    return nc


# kernel pixel index p = r*64 + par*32 + wq  <->  rot pixel r*64 + 2*wq + par
_r = np.arange(N)
_PERM = (_r // 64) * 64 + 2 * (_r % 32) + ((_r // 32) % 2)


def _prep_maps(x, y, W_self1, b_self1, W_self2, b_self2, W_x1, b_x1, W_x2,
               b_x2, W_y1, b_y1, W_y2, b_y2, W_up, b_up):
    f64 = np.float64
    d = lambda a: np.asarray(a, f64)

    Wx = d(W_x2) @ d(W_x1); bx = d(W_x2) @ d(b_x1) + d(b_x2)
    Wy = d(W_y2) @ d(W_y1); by = d(W_y2) @ d(b_y1) + d(b_y2)
    Ws = d(W_self2) @ d(W_self1); bs = d(W_self2) @ d(b_self1) + d(b_self2)
    bup2 = d(b_up) + d(W_up) @ bs

    def fold_t(Wm, nchunk):
        return np.ascontiguousarray(
            Wm.T.reshape(nchunk, 128, 64).transpose(1, 0, 2).reshape(128, nchunk * 64))

    w16 = np.zeros((128, 768), np.float16)
    w16[:, 0:128] = fold_t(Wx, 2).astype(np.float16)
    w16[:, 128:256] = fold_t(Ws, 2).astype(np.float16)
    w16[:, 256:512] = fold_t(Wy, 4).astype(np.float16)
    w16[0:65, 512:768] = np.concatenate(
        [bup2.reshape(1, 256), d(W_up).T], axis=0).astype(np.float16)
    bxy = np.stack([bx, by], axis=1).astype(np.float32)

    maps = []
    for b in range(B):
        xf = np.asarray(x[b], np.float16).reshape(CX, N)
        yf = np.asarray(y[b], np.float16)          # [512, 32, 32]
        for half in range(2):
            rot = (np.arange(N) + half * NH) % N
            xr = xf[:, rot][:, _PERM]
            xs_h = np.ascontiguousarray(
                xr.reshape(2, 128, 8, 512).transpose(1, 2, 0, 3).reshape(128, 8192))
            yrot = (np.arange(32) + half * 16) % 32
            yr = yf[:, yrot, :].reshape(CY, 1024)
            yb_h = np.ascontiguousarray(
                yr.reshape(4, 128, 2, 512).transpose(1, 2, 0, 3).reshape(128, 4096))
            if half == 0:
                pre_g, post_g, sa_g, sb_g = 0, 31, 16, 15
            else:
                pre_g, post_g, sa_g, sb_g = 15, 16, 31, 0
            ypad = np.stack([yf[:, pre_g, :], yf[:, post_g, :],
                             yf[:, sa_g, :], yf[:, sb_g, :]], axis=1)  # [512,4,32]
            ybp_h = np.ascontiguousarray(
                ypad.reshape(4, 128, 128).transpose(1, 0, 2).reshape(128, 512))
            maps.append({"xs16": xs_h, "yb16": yb_h, "ybp16": ybp_h,
                         "w16": w16, "bxy": bxy})
    return maps


def _run(inputs, trace=False, trace_kwargs=None, debug=False):
    key = ("nc", debug)
    if key not in _CACHE:
        _CACHE[key] = _build(debug=debug)
    nc = _CACHE[key]
    maps = _prep_maps(**inputs)
    res = run_bass_kernel_spmd(nc, maps, list(range(8)), trace=trace,
                               **(trace_kwargs or {}))
    outs = np.empty((B, CX, H, W), np.float32)
    inv = np.empty(N, np.int64)
    inv[_PERM] = np.arange(N)          # kernel col for rot pixel j: inv[j]
    for b in range(B):
        for half in range(2):
            o = res.results[2 * b + half]["out"]                # [128, 4096]
            oh = o.reshape(128, 2, NH).transpose(1, 0, 2).reshape(CX, NH)
            oh = oh[:, inv[0:NH]]      # un-permute w-deinterleave
            outs[b, :, :, :].reshape(CX, N)[:, half * NH:(half + 1) * NH] = oh
    return outs, res


def kernel(**inputs):
    outs, _ = _run(inputs, trace=False)
    return outs
